# revision 12
# baseline (speedup 1.0000x reference)
"""Trainium2 Bass kernel for DescriptorMatcher (mutual nearest neighbor matching).

Problem: given desc0 [B,N,D], desc1 [B,M,D] (B=4, N=M=8192, D=128, fp32):
    sim     = desc0 @ desc1^T                      [B,N,M]
    score0  = max_m sim                            [B,N]
    match01 = argmax_m sim                         [B,N]
    match10 = argmax_n sim                         [B,M]
    valid   = (match10[match01[n]] == n) & (score0 > 0.1)
returns (match01, score0, valid).

Key reformulation: the mutual check never needs match10 indices:
    match10[match01[n]] == n  <=>  sim[n, match01[n]] == colmax[match01[n]]
                              <=>  score0[n] == colmax[match01[n]]
(exact fp32 equality is safe because both sides are max-chains over the
same on-device fp32 values; max is exact).

Sharding: 8 cores = 4 batches x 2 row-halves. Each core computes, for its
4096-row slab of one batch:
  - score0 / match01 for its rows (exact, fp32 matmul + max8/max_index)
  - partial column max over its rows [8192]
Host glue: pairwise max of the two partial colmax vectors per batch, then
valid = (score0 > 0.1) & (score0 == colmax[match01]).

Per-core kernel structure (Tile framework):
  for each of 32 n-tiles (128 rows):
    PE:  16 fp32 matmuls  ([128d,128n]^T x [128d,512m] -> PSUM [128,2048] x4)
    ACT: copy each PSUM chunk -> SBUF row buffer [128, 8192]
    DVE: colacc = max(colacc, row)   (column-side accumulate)
    DVE: max8 + max_index on row     (row max + exact first-occurrence argmax)
  GPSIMD: partition_all_reduce(max) over colacc -> partial colmax [8192]

VectorE is the bottleneck at ~97% occupancy (three 1-elem/cycle passes per
tile); fp32 matmuls hide completely under it, so reduced-precision matmul
modes would buy nothing while costing exactness.
"""

import numpy as np

import concourse.bass as bass  # noqa: F401  (bass must import before tile)
import concourse.mybir as mybir
import concourse.tile as tile
from concourse import bacc, bass_isa

B, N, M, D = 4, 8192, 8192, 128
NCORES = 8
HALF = N // 2          # rows per core
NT = HALF // 128       # 32 n-tiles per core
CW = 1024              # input-DMA chunk width


def _build():
    f32 = mybir.dt.float32
    u32 = mybir.dt.uint32
    nc = bacc.Bacc("TRN2", target_bir_lowering=False, debug=False,
                   num_devices=NCORES)
    at = nc.dram_tensor("at", [D, HALF], f32, kind="ExternalInput").ap()
    bt = nc.dram_tensor("bt", [D, M], f32, kind="ExternalInput").ap()
    score_o = nc.dram_tensor("score", [128, NT * 8], f32, kind="ExternalOutput").ap()
    match_o = nc.dram_tensor("match", [128, NT * 8], u32, kind="ExternalOutput").ap()
    colp_o = nc.dram_tensor("colp", [1, M], f32, kind="ExternalOutput").ap()

    with tile.TileContext(nc) as tc:
        with tc.tile_pool(name="big", bufs=1) as big, \
             tc.tile_pool(name="rows", bufs=3) as rows, \
             tc.tile_pool(name="ps", bufs=2, space="PSUM") as ps:
            atb = big.tile([128, HALF], f32, name="atb")
            btb = big.tile([128, M], f32, name="btb")
            # chunked loads so the first matmuls start before the full 6 MB
            # of descriptors has landed
            for c in range(0, HALF, CW):
                nc.sync.dma_start(atb[:, c:c + CW], at[:, c:c + CW])
            for c in range(0, M, CW):
                nc.sync.dma_start(btb[:, c:c + CW], bt[:, c:c + CW])

            colacc = big.tile([128, M], f32, name="colacc")
            # max8/max_index outs land directly in these batched buffers:
            # column t*8+k holds the k-th of tile t's top-8; host reads ::8
            score8 = big.tile([128, NT * 8], f32, name="score8")
            match8 = big.tile([128, NT * 8], u32, name="match8")

            for t in range(NT):
                row = rows.tile([128, M], f32, tag="row", name="row")
                for c in range(4):
                    pt = ps.tile([128, 2048], f32, tag="pt", name="pt")
                    for j in range(4):
                        mlo = c * 2048 + j * 512
                        nc.tensor.matmul(
                            pt[:, j * 512:(j + 1) * 512],
                            atb[:, t * 128:(t + 1) * 128],
                            btb[:, mlo:mlo + 512],
                            start=True, stop=True)
                    nc.scalar.copy(row[:, c * 2048:(c + 1) * 2048], pt[:])
                # column-side: running elementwise max over n-tiles
                if t == 0:
                    # per-chunk copies start as soon as each ACT copy lands
                    for c in range(4):
                        nc.vector.tensor_copy(colacc[:, c * 2048:(c + 1) * 2048],
                                              row[:, c * 2048:(c + 1) * 2048])
                else:
                    nc.vector.tensor_tensor(colacc[:], colacc[:], row[:],
                                            op=mybir.AluOpType.max)
                # row-side: top-8 values then first-occurrence index of the max
                nc.vector.max(score8[:, t * 8:(t + 1) * 8], row[:])
                nc.vector.max_index(match8[:, t * 8:(t + 1) * 8],
                                    score8[:, t * 8:(t + 1) * 8], row[:])

            # partial column max over this core's 4096 rows
            cp = rows.tile([128, M], f32, tag="row", name="cp")
            nc.gpsimd.partition_all_reduce(cp[:], colacc[:], channels=128,
                                           reduce_op=bass_isa.ReduceOp.max)
            nc.sync.dma_start(score_o[:], score8[:])
            nc.sync.dma_start(match_o[:], match8[:])
            nc.sync.dma_start(colp_o[:], cp[0:1, :])
    nc.compile()
    return nc


_cached_exec = None


def _build_exec():
    """Compile the NEFF once and return a cached 8-core jitted executable."""
    import jax
    from jax.sharding import Mesh, PartitionSpec
    from jax.experimental.shard_map import shard_map
    from concourse import bass2jax
    from concourse.bass2jax import _bass_exec_p, install_neuronx_cc_hook

    install_neuronx_cc_hook()
    nc = _build()

    partition_name = nc.partition_id_tensor.name if nc.partition_id_tensor else None
    in_names, out_names, out_avals, out_shapes = [], [], [], []
    for alloc in nc.m.functions[0].allocations:
        if not isinstance(alloc, mybir.MemoryLocationSet):
            continue
        name = alloc.memorylocations[0].name
        if alloc.kind == "ExternalInput":
            if name != partition_name:
                in_names.append(name)
        elif alloc.kind == "ExternalOutput":
            shape = tuple(alloc.tensor_shape)
            dtype = mybir.dt.np(alloc.dtype)
            out_names.append(name)
            out_shapes.append((shape, dtype))
            out_avals.append(jax.core.ShapedArray(shape, dtype))
    n_params = len(in_names)
    n_outs = len(out_names)
    all_in_names = in_names + out_names
    if partition_name is not None:
        all_in_names = all_in_names + [partition_name]

    def _body(*args):
        operands = list(args)
        if partition_name is not None:
            operands.append(bass2jax.partition_id_tensor())
        outs = _bass_exec_p.bind(
            *operands, out_avals=tuple(out_avals), in_names=tuple(all_in_names),
            out_names=tuple(out_names), lowering_input_output_aliases=(),
            sim_require_finite=True, sim_require_nnan=True, nc=nc)
        return tuple(outs)

    devices = jax.devices()[:NCORES]
    mesh = Mesh(np.asarray(devices), ("core",))
    in_specs = (PartitionSpec("core"),) * (n_params + n_outs)
    out_specs = (PartitionSpec("core"),) * n_outs
    sharded = jax.jit(
        shard_map(_body, mesh=mesh, in_specs=in_specs, out_specs=out_specs,
                  check_rep=False),
        keep_unused=True)
    return {
        "nc": nc, "fn": sharded, "in_names": in_names,
        "out_names": out_names, "out_shapes": out_shapes,
    }


def kernel(desc0, desc1):
    global _cached_exec
    desc0 = np.asarray(desc0, dtype=np.float32)
    desc1 = np.asarray(desc1, dtype=np.float32)
    assert desc0.shape == (B, N, D) and desc1.shape == (B, M, D)

    if _cached_exec is None:
        _cached_exec = _build_exec()
    ex = _cached_exec

    # build concatenated per-core inputs: axis 0 stacks the 8 cores
    # core = 2*b + h handles rows [h*4096, (h+1)*4096) of batch b
    at_all = np.concatenate(
        [desc0[b, h * HALF:(h + 1) * HALF].T for b in range(B) for h in range(2)],
        axis=0)                                             # [8*128, 4096]
    bt_all = np.concatenate(
        [desc1[b].T for b in range(B) for h in range(2)], axis=0)  # [8*128, 8192]
    ins = {"at": np.ascontiguousarray(at_all), "bt": np.ascontiguousarray(bt_all)}
    concat_in = [ins[n] for n in ex["in_names"]]
    concat_zeros = [np.zeros((NCORES * s[0], *s[1:]), dt)
                    for (s, dt) in ex["out_shapes"]]

    out_arrs = ex["fn"](*concat_in, *concat_zeros)
    res = {}
    for i, name in enumerate(ex["out_names"]):
        shape, dt = ex["out_shapes"][i]
        res[name] = np.asarray(out_arrs[i]).reshape(NCORES, *shape)

    match01 = np.empty((B, N), dtype=np.int32)
    score0 = np.empty((B, N), dtype=np.float32)
    valid = np.empty((B, N), dtype=bool)
    colp = res["colp"].reshape(B, 2, M)
    colmax = colp.max(axis=1)                               # [B, M]

    for core in range(NCORES):
        b, h = divmod(core, 2)
        # score/match stored [partition p, tile t * 8 + k]; k=0 is the top-1
        # row n = t*128 + p
        s = res["score"][core][:, ::8].T.reshape(-1)        # [4096]
        m = res["match"][core][:, ::8].T.reshape(-1).astype(np.int64)
        sl = slice(h * HALF, (h + 1) * HALF)
        score0[b, sl] = s
        match01[b, sl] = m.astype(np.int32)
        valid[b, sl] = (s > 0.1) & (s == colmax[b][m])

    return match01, score0, valid


# revision 13
# speedup vs baseline: 1.2011x; 1.2011x over previous
"""Trainium2 Bass kernel for DescriptorMatcher (mutual nearest neighbor matching).

Problem: given desc0 [B,N,D], desc1 [B,M,D] (B=4, N=M=8192, D=128, fp32):
    sim     = desc0 @ desc1^T                      [B,N,M]
    score0  = max_m sim                            [B,N]
    match01 = argmax_m sim                         [B,N]
    match10 = argmax_n sim                         [B,M]
    valid   = (match10[match01[n]] == n) & (score0 > 0.1)
returns (match01, score0, valid).

Key reformulation: the mutual check never needs match10 indices:
    match10[match01[n]] == n  <=>  score0[n] == colmax[match01[n]]
(exact fp32 equality is safe: both sides are max-chains over the same
on-device fp32 values, and max is exact).

Sharding: 8 cores = 4 batches x 2 row-halves (4096 rows each).

Two-phase execution (VectorE is the bottleneck; this needs 2 full VectorE
passes per element instead of 3):

Phase 1 (per core, ~590 us): for each of 32 n-tiles:
    PE:  16 fp32 matmuls -> PSUM [128,2048] x4
    ACT: copy PSUM -> SBUF row buffer [128, 8192]
    DVE: colacc = max(colacc, row)                      (column side)
    DVE: one fused reduce [128,8,1024]-view -> CM[t]    (8 chunk maxima/row)
  then partition_all_reduce(max) -> partial colmax [8192].
  Host: score0 = CM.max(1); c* = CM.argmax(1) (first occurrence); groups
  rows by winning chunk.

Phase 2 (per core, ~125 us): for each group (rows sharing winning chunk c),
  recompute sim[:, c*1024:(c+1)*1024] with identically-laid-out fp32 matmuls
  (bit-exact: each PE output element depends only on its own lhsT/rhs column
  pair), then max_index(score, chunk) gives the exact first-occurrence
  within-chunk position. match01 = c*1024 + within.

Rows overflowing a group's padded capacity (impossible for anything
gaussian-like; needs >768 of 4096 rows sharing one winning chunk) fall back
to a host-side recompute of that row.
"""

import numpy as np

import concourse.bass as bass  # noqa: F401  (bass must import before tile)
import concourse.mybir as mybir
import concourse.tile as tile
from concourse import bacc, bass_isa

B, N, M, D = 4, 8192, 8192, 128
NCORES = 8
HALF = N // 2          # rows per core
NT = HALF // 128       # 32 n-tiles per core
CW = 1024              # input-DMA chunk width
PAD = 768              # phase-2 rows per chunk-group (+12 sigma of binomial)
NST = 8 * PAD // 128   # 48 phase-2 sub-tiles


def _build1():
    f32 = mybir.dt.float32
    nc = bacc.Bacc("TRN2", target_bir_lowering=False, debug=False,
                   num_devices=NCORES)
    at = nc.dram_tensor("at", [D, HALF], f32, kind="ExternalInput").ap()
    bt = nc.dram_tensor("bt", [D, M], f32, kind="ExternalInput").ap()
    cm_o = nc.dram_tensor("cm", [128, NT * 8], f32, kind="ExternalOutput").ap()
    colp_o = nc.dram_tensor("colp", [1, M], f32, kind="ExternalOutput").ap()

    with tile.TileContext(nc) as tc:
        with tc.tile_pool(name="big", bufs=1) as big, \
             tc.tile_pool(name="rows", bufs=3) as rows, \
             tc.tile_pool(name="ps", bufs=2, space="PSUM") as ps:
            atb = big.tile([128, HALF], f32, name="atb")
            btb = big.tile([128, M], f32, name="btb")
            for c in range(0, HALF, CW):
                nc.sync.dma_start(atb[:, c:c + CW], at[:, c:c + CW])
            for c in range(0, M, CW):
                nc.sync.dma_start(btb[:, c:c + CW], bt[:, c:c + CW])
            colacc = big.tile([128, M], f32, name="colacc")
            cm_all = big.tile([128, NT * 8], f32, name="cm_all")
            for t in range(NT):
                row = rows.tile([128, M], f32, tag="row", name="row")
                for c in range(4):
                    pt = ps.tile([128, 2048], f32, tag="pt", name="pt")
                    for j in range(4):
                        mlo = c * 2048 + j * 512
                        nc.tensor.matmul(pt[:, j * 512:(j + 1) * 512],
                                         atb[:, t * 128:(t + 1) * 128],
                                         btb[:, mlo:mlo + 512],
                                         start=True, stop=True)
                    nc.scalar.copy(row[:, c * 2048:(c + 1) * 2048], pt[:])
                if t == 0:
                    for c in range(4):
                        nc.vector.tensor_copy(colacc[:, c * 2048:(c + 1) * 2048],
                                              row[:, c * 2048:(c + 1) * 2048])
                else:
                    nc.vector.tensor_tensor(colacc[:], colacc[:], row[:],
                                            op=mybir.AluOpType.max)
                v = row[:].rearrange("p (c w) -> p c w", w=1024)
                nc.vector.tensor_reduce(cm_all[:, t * 8:(t + 1) * 8], v,
                                        axis=mybir.AxisListType.X,
                                        op=mybir.AluOpType.max)
            cp = rows.tile([128, M], f32, tag="row", name="cp")
            nc.gpsimd.partition_all_reduce(cp[:], colacc[:], channels=128,
                                           reduce_op=bass_isa.ReduceOp.max)
            nc.sync.dma_start(cm_o[:], cm_all[:])
            nc.sync.dma_start(colp_o[:], cp[0:1, :])
    nc.compile()
    return nc


def _build2():
    f32, u32 = mybir.dt.float32, mybir.dt.uint32
    nc = bacc.Bacc("TRN2", target_bir_lowering=False, debug=False,
                   num_devices=NCORES)
    at2 = nc.dram_tensor("at2", [D, 8 * PAD], f32, kind="ExternalInput").ap()
    bt = nc.dram_tensor("bt", [D, M], f32, kind="ExternalInput").ap()
    sg = nc.dram_tensor("sg", [128, NST], f32, kind="ExternalInput").ap()
    idx_o = nc.dram_tensor("idx", [128, NST * 8], u32, kind="ExternalOutput").ap()
    with tile.TileContext(nc) as tc:
        with tc.tile_pool(name="big", bufs=1) as big, \
             tc.tile_pool(name="work", bufs=3) as work, \
             tc.tile_pool(name="ps", bufs=4, space="PSUM") as ps:
            a2b = big.tile([128, 8 * PAD], f32, name="a2b")
            btb = big.tile([128, M], f32, name="btb")
            sgb = big.tile([128, NST], f32, name="sgb")
            nc.sync.dma_start(a2b[:], at2[:])
            nc.sync.dma_start(btb[:], bt[:])
            nc.sync.dma_start(sgb[:], sg[:])
            idx8 = big.tile([128, NST * 8], u32, name="idx8")
            KP = PAD // 128
            for g in range(8):
                for k in range(KP):
                    st = g * KP + k
                    pt = ps.tile([128, 1024], f32, tag="pt", name="pt")
                    for j in range(2):
                        nc.tensor.matmul(
                            pt[:, j * 512:(j + 1) * 512],
                            a2b[:, st * 128:(st + 1) * 128],
                            btb[:, g * 1024 + j * 512: g * 1024 + (j + 1) * 512],
                            start=True, stop=True)
                    ch = work.tile([128, 1024], f32, tag="ch", name="ch")
                    nc.scalar.copy(ch[:], pt[:])
                    sc8 = work.tile([128, 8], f32, tag="sc8", name="sc8")
                    nc.vector.tensor_copy(
                        sc8[:], sgb[:, st:st + 1].to_broadcast((128, 8)))
                    nc.vector.max_index(idx8[:, st * 8:(st + 1) * 8],
                                        sc8[:], ch[:])
            nc.sync.dma_start(idx_o[:], idx8[:])
    nc.compile()
    return nc


_cached = None


def _make_exec(nc):
    import jax
    from jax.sharding import Mesh, PartitionSpec
    from jax.experimental.shard_map import shard_map
    from concourse import bass2jax
    from concourse.bass2jax import _bass_exec_p

    partition_name = nc.partition_id_tensor.name if nc.partition_id_tensor else None
    in_names, out_names, out_avals, out_shapes = [], [], [], []
    for alloc in nc.m.functions[0].allocations:
        if not isinstance(alloc, mybir.MemoryLocationSet):
            continue
        name = alloc.memorylocations[0].name
        if alloc.kind == "ExternalInput":
            if name != partition_name:
                in_names.append(name)
        elif alloc.kind == "ExternalOutput":
            shape = tuple(alloc.tensor_shape)
            dtype = mybir.dt.np(alloc.dtype)
            out_names.append(name)
            out_shapes.append((shape, dtype))
            out_avals.append(jax.core.ShapedArray(shape, dtype))
    n_params = len(in_names)
    n_outs = len(out_names)
    all_in_names = in_names + out_names
    if partition_name is not None:
        all_in_names = all_in_names + [partition_name]

    def _body(*args):
        operands = list(args)
        if partition_name is not None:
            operands.append(bass2jax.partition_id_tensor())
        outs = _bass_exec_p.bind(
            *operands, out_avals=tuple(out_avals), in_names=tuple(all_in_names),
            out_names=tuple(out_names), lowering_input_output_aliases=(),
            sim_require_finite=True, sim_require_nnan=True, nc=nc)
        return tuple(outs)

    devices = jax.devices()[:NCORES]
    mesh = Mesh(np.asarray(devices), ("core",))
    in_specs = (PartitionSpec("core"),) * (n_params + n_outs)
    out_specs = (PartitionSpec("core"),) * n_outs
    fn = jax.jit(shard_map(_body, mesh=mesh, in_specs=in_specs,
                           out_specs=out_specs, check_rep=False),
                 keep_unused=True)
    return {"fn": fn, "in_names": in_names, "out_names": out_names,
            "out_shapes": out_shapes, "nc": nc}


def _run(ex, ins):
    """ins: dict name -> [NCORES, *shape]; returns dict name -> [NCORES, *shape]."""
    concat_in = [np.ascontiguousarray(ins[n].reshape(-1, *ins[n].shape[2:]))
                 for n in ex["in_names"]]
    concat_zeros = [np.zeros((NCORES * s[0], *s[1:]), dt)
                    for (s, dt) in ex["out_shapes"]]
    out_arrs = ex["fn"](*concat_in, *concat_zeros)
    return {name: np.asarray(out_arrs[i]).reshape(NCORES, *ex["out_shapes"][i][0])
            for i, name in enumerate(ex["out_names"])}


def kernel(desc0, desc1):
    global _cached
    desc0 = np.asarray(desc0, dtype=np.float32)
    desc1 = np.asarray(desc1, dtype=np.float32)
    assert desc0.shape == (B, N, D) and desc1.shape == (B, M, D)

    if _cached is None:
        _cached = (_make_exec(_build1()), _make_exec(_build2()))
    ex1, ex2 = _cached

    a_slab = np.stack([desc0[b, h * HALF:(h + 1) * HALF]
                       for b in range(B) for h in range(2)])      # [8,4096,128]
    bt_all = np.stack([desc1[b].transpose(1, 0)
                       for b in range(B) for h in range(2)])      # [8,128,8192]
    at_all = a_slab.transpose(0, 2, 1)                            # [8,128,4096]

    r1 = _run(ex1, {"at": at_all, "bt": bt_all})

    # host glue: score/chunk-argmax + grouping for phase 2
    cm = r1["cm"].reshape(NCORES, 128, NT, 8).transpose(0, 2, 1, 3) \
                 .reshape(NCORES, HALF, 8)
    score0_c = cm.max(axis=2)                                     # [8, 4096]
    cstar_c = cm.argmax(axis=2)                                   # [8, 4096]

    at2 = np.zeros((NCORES, D, 8 * PAD), np.float32)
    sg = np.full((NCORES, 128, NST), 1e30, np.float32)
    slot_of_row = np.full((NCORES, HALF), -1, np.int64)
    overflow = []                                                 # (core, row)
    for core in range(NCORES):
        for g in range(8):
            rows = np.nonzero(cstar_c[core] == g)[0]
            if len(rows) > PAD:
                overflow.extend((core, r) for r in rows[PAD:])
                rows = rows[:PAD]
            slots = g * PAD + np.arange(len(rows))
            slot_of_row[core, rows] = slots
            at2[core][:, slots] = a_slab[core][rows].T
            sg[core][slots % 128, slots // 128] = score0_c[core][rows]

    r2 = _run(ex2, {"at2": at2, "bt": bt_all, "sg": sg})
    within = r2["idx"][:, :, ::8]                                 # [8, 128, NST]

    match01 = np.empty((B, N), dtype=np.int32)
    score0 = np.empty((B, N), dtype=np.float32)
    valid = np.empty((B, N), dtype=bool)
    colmax = r1["colp"].reshape(B, 2, M).max(axis=1)              # [B, M]

    for core in range(NCORES):
        b, h = divmod(core, 2)
        s = score0_c[core]
        sl = slot_of_row[core]
        m = cstar_c[core] * 1024 + \
            within[core][sl % 128, sl // 128].astype(np.int64)
        sel = slice(h * HALF, (h + 1) * HALF)
        score0[b, sel] = s
        match01[b, sel] = m.astype(np.int32)
        valid[b, sel] = (s > 0.1) & (s == colmax[b][m])

    for core, row in overflow:                                    # ~never taken
        b, h = divmod(core, 2)
        simrow = a_slab[core][row] @ desc1[b].T
        n = h * HALF + row
        match01[b, n] = int(simrow.argmax())
        score0[b, n] = simrow.max()
        valid[b, n] = (score0[b, n] > 0.1) & \
                      (score0[b, n] == colmax[b][match01[b, n]])

    return match01, score0, valid


# revision 15
# speedup vs baseline: 1.2504x; 1.0411x over previous
"""Trainium2 Bass kernel for DescriptorMatcher (mutual nearest neighbor matching).

Problem: given desc0 [B,N,D], desc1 [B,M,D] (B=4, N=M=8192, D=128, fp32):
    sim     = desc0 @ desc1^T                      [B,N,M]
    score0  = max_m sim                            [B,N]
    match01 = argmax_m sim                         [B,N]
    match10 = argmax_n sim                         [B,M]
    valid   = (match10[match01[n]] == n) & (score0 > 0.1)
returns (match01, score0, valid).

Key reformulation: the mutual check never needs match10 indices:
    match10[match01[n]] == n  <=>  score0[n] == colmax[match01[n]]
(exact fp32 equality is safe: both sides are max-chains over the same
on-device fp32 values, and max is exact).

Sharding: 8 cores = 4 batches x 2 row-halves (4096 rows each).

Two-phase execution (VectorE is the bottleneck; this needs 2 full VectorE
passes per element instead of 3):

Phase 1 (per core, ~590 us): for each of 32 n-tiles:
    PE:  16 fp32 matmuls -> PSUM [128,2048] x4
    ACT: copy PSUM -> SBUF row buffer [128, 8192]
    DVE: colacc = max(colacc, row)                      (column side)
    DVE: one fused reduce [128,8,1024]-view -> CM[t]    (8 chunk maxima/row)
  then partition_all_reduce(max) -> partial colmax [8192].
  Host: score0 = CM.max(1); c* = CM.argmax(1) (first occurrence); groups
  rows by winning chunk.

Phase 2 (per core, ~125 us): for each group (rows sharing winning chunk c),
  recompute sim[:, c*1024:(c+1)*1024] with identically-laid-out fp32 matmuls
  (bit-exact: each PE output element depends only on its own lhsT/rhs column
  pair), then max_index(score, chunk) gives the exact first-occurrence
  within-chunk position. match01 = c*1024 + within.

Rows overflowing a group's padded capacity (impossible for anything
gaussian-like; needs >768 of 4096 rows sharing one winning chunk) fall back
to a host-side recompute of that row.
"""

import numpy as np

import concourse.bass as bass  # noqa: F401  (bass must import before tile)
import concourse.mybir as mybir
import concourse.tile as tile
from concourse import bacc, bass_isa

B, N, M, D = 4, 8192, 8192, 128
NCORES = 8
HALF = N // 2          # rows per core
NT = HALF // 128       # 32 n-tiles per core
CW = 1024              # input-DMA chunk width
PAD = 640              # phase-2 rows per chunk-group (+6 sigma of binomial;
                       # overflow degrades to host fallback, never wrong)
NST = 8 * PAD // 128   # 48 phase-2 sub-tiles


def _build1():
    f32 = mybir.dt.float32
    nc = bacc.Bacc("TRN2", target_bir_lowering=False, debug=False,
                   num_devices=NCORES)
    at = nc.dram_tensor("at", [D, HALF], f32, kind="ExternalInput").ap()
    bt = nc.dram_tensor("bt", [D, M], f32, kind="ExternalInput").ap()
    cm_o = nc.dram_tensor("cm", [128, NT * 8], f32, kind="ExternalOutput").ap()
    colp_o = nc.dram_tensor("colp", [1, M], f32, kind="ExternalOutput").ap()

    with tile.TileContext(nc) as tc:
        with tc.tile_pool(name="big", bufs=1) as big, \
             tc.tile_pool(name="rows", bufs=3) as rows, \
             tc.tile_pool(name="ps", bufs=2, space="PSUM") as ps:
            atb = big.tile([128, HALF], f32, name="atb")
            btb = big.tile([128, M], f32, name="btb")
            for c in range(0, HALF, CW):
                nc.sync.dma_start(atb[:, c:c + CW], at[:, c:c + CW])
            for c in range(0, M, CW):
                nc.sync.dma_start(btb[:, c:c + CW], bt[:, c:c + CW])
            colacc = big.tile([128, M], f32, name="colacc")
            cm_all = big.tile([128, NT * 8], f32, name="cm_all")
            for t in range(NT):
                row = rows.tile([128, M], f32, tag="row", name="row")
                for c in range(4):
                    pt = ps.tile([128, 2048], f32, tag="pt", name="pt")
                    for j in range(4):
                        mlo = c * 2048 + j * 512
                        nc.tensor.matmul(pt[:, j * 512:(j + 1) * 512],
                                         atb[:, t * 128:(t + 1) * 128],
                                         btb[:, mlo:mlo + 512],
                                         start=True, stop=True)
                    nc.scalar.copy(row[:, c * 2048:(c + 1) * 2048], pt[:])
                if t == 0:
                    for c in range(4):
                        nc.vector.tensor_copy(colacc[:, c * 2048:(c + 1) * 2048],
                                              row[:, c * 2048:(c + 1) * 2048])
                else:
                    nc.vector.tensor_tensor(colacc[:], colacc[:], row[:],
                                            op=mybir.AluOpType.max)
                v = row[:].rearrange("p (c w) -> p c w", w=1024)
                nc.vector.tensor_reduce(cm_all[:, t * 8:(t + 1) * 8], v,
                                        axis=mybir.AxisListType.X,
                                        op=mybir.AluOpType.max)
            cp = rows.tile([128, M], f32, tag="row", name="cp")
            nc.gpsimd.partition_all_reduce(cp[:], colacc[:], channels=128,
                                           reduce_op=bass_isa.ReduceOp.max)
            nc.sync.dma_start(cm_o[:], cm_all[:])
            nc.sync.dma_start(colp_o[:], cp[0:1, :])
    nc.compile()
    return nc


def _build2():
    f32, u32 = mybir.dt.float32, mybir.dt.uint32
    nc = bacc.Bacc("TRN2", target_bir_lowering=False, debug=False,
                   num_devices=NCORES)
    at2 = nc.dram_tensor("at2", [D, 8 * PAD], f32, kind="ExternalInput").ap()
    bt = nc.dram_tensor("bt", [D, M], f32, kind="ExternalInput").ap()
    sg = nc.dram_tensor("sg", [128, NST], f32, kind="ExternalInput").ap()
    idx_o = nc.dram_tensor("idx", [128, NST * 8], u32, kind="ExternalOutput").ap()
    with tile.TileContext(nc) as tc:
        with tc.tile_pool(name="big", bufs=1) as big, \
             tc.tile_pool(name="work", bufs=3) as work, \
             tc.tile_pool(name="ps", bufs=4, space="PSUM") as ps:
            a2b = big.tile([128, 8 * PAD], f32, name="a2b")
            btb = big.tile([128, M], f32, name="btb")
            sgb = big.tile([128, NST], f32, name="sgb")
            nc.sync.dma_start(sgb[:], sg[:])
            # chunked so group 0's matmuls start before all input has landed
            for c in range(0, 8 * PAD, PAD):
                nc.sync.dma_start(a2b[:, c:c + PAD], at2[:, c:c + PAD])
            for c in range(0, M, CW):
                nc.sync.dma_start(btb[:, c:c + CW], bt[:, c:c + CW])
            idx8 = big.tile([128, NST * 8], u32, name="idx8")
            KP = PAD // 128
            for g in range(8):
                for k in range(KP):
                    st = g * KP + k
                    pt = ps.tile([128, 1024], f32, tag="pt", name="pt")
                    for j in range(2):
                        nc.tensor.matmul(
                            pt[:, j * 512:(j + 1) * 512],
                            a2b[:, st * 128:(st + 1) * 128],
                            btb[:, g * 1024 + j * 512: g * 1024 + (j + 1) * 512],
                            start=True, stop=True)
                    ch = work.tile([128, 1024], f32, tag="ch", name="ch")
                    nc.scalar.copy(ch[:], pt[:])
                    sc8 = work.tile([128, 8], f32, tag="sc8", name="sc8")
                    nc.vector.tensor_copy(
                        sc8[:], sgb[:, st:st + 1].to_broadcast((128, 8)))
                    nc.vector.max_index(idx8[:, st * 8:(st + 1) * 8],
                                        sc8[:], ch[:])
            nc.sync.dma_start(idx_o[:], idx8[:])
    nc.compile()
    return nc


_cached = None


def _make_exec(nc):
    import jax
    from jax.sharding import Mesh, PartitionSpec
    from jax.experimental.shard_map import shard_map
    from concourse import bass2jax
    from concourse.bass2jax import _bass_exec_p

    partition_name = nc.partition_id_tensor.name if nc.partition_id_tensor else None
    in_names, out_names, out_avals, out_shapes = [], [], [], []
    for alloc in nc.m.functions[0].allocations:
        if not isinstance(alloc, mybir.MemoryLocationSet):
            continue
        name = alloc.memorylocations[0].name
        if alloc.kind == "ExternalInput":
            if name != partition_name:
                in_names.append(name)
        elif alloc.kind == "ExternalOutput":
            shape = tuple(alloc.tensor_shape)
            dtype = mybir.dt.np(alloc.dtype)
            out_names.append(name)
            out_shapes.append((shape, dtype))
            out_avals.append(jax.core.ShapedArray(shape, dtype))
    n_params = len(in_names)
    n_outs = len(out_names)
    all_in_names = in_names + out_names
    if partition_name is not None:
        all_in_names = all_in_names + [partition_name]

    def _body(*args):
        operands = list(args)
        if partition_name is not None:
            operands.append(bass2jax.partition_id_tensor())
        outs = _bass_exec_p.bind(
            *operands, out_avals=tuple(out_avals), in_names=tuple(all_in_names),
            out_names=tuple(out_names), lowering_input_output_aliases=(),
            sim_require_finite=True, sim_require_nnan=True, nc=nc)
        return tuple(outs)

    devices = jax.devices()[:NCORES]
    mesh = Mesh(np.asarray(devices), ("core",))
    in_specs = (PartitionSpec("core"),) * (n_params + n_outs)
    out_specs = (PartitionSpec("core"),) * n_outs
    fn = jax.jit(shard_map(_body, mesh=mesh, in_specs=in_specs,
                           out_specs=out_specs, check_rep=False),
                 keep_unused=True)
    return {"fn": fn, "in_names": in_names, "out_names": out_names,
            "out_shapes": out_shapes, "nc": nc}


def _run(ex, ins):
    """ins: dict name -> [NCORES, *shape]; returns dict name -> [NCORES, *shape]."""
    concat_in = [np.ascontiguousarray(ins[n].reshape(-1, *ins[n].shape[2:]))
                 for n in ex["in_names"]]
    concat_zeros = [np.zeros((NCORES * s[0], *s[1:]), dt)
                    for (s, dt) in ex["out_shapes"]]
    out_arrs = ex["fn"](*concat_in, *concat_zeros)
    return {name: np.asarray(out_arrs[i]).reshape(NCORES, *ex["out_shapes"][i][0])
            for i, name in enumerate(ex["out_names"])}


def kernel(desc0, desc1):
    global _cached
    desc0 = np.asarray(desc0, dtype=np.float32)
    desc1 = np.asarray(desc1, dtype=np.float32)
    assert desc0.shape == (B, N, D) and desc1.shape == (B, M, D)

    if _cached is None:
        _cached = (_make_exec(_build1()), _make_exec(_build2()))
    ex1, ex2 = _cached

    a_slab = np.stack([desc0[b, h * HALF:(h + 1) * HALF]
                       for b in range(B) for h in range(2)])      # [8,4096,128]
    bt_all = np.stack([desc1[b].transpose(1, 0)
                       for b in range(B) for h in range(2)])      # [8,128,8192]
    at_all = a_slab.transpose(0, 2, 1)                            # [8,128,4096]

    r1 = _run(ex1, {"at": at_all, "bt": bt_all})

    # host glue: score/chunk-argmax + grouping for phase 2
    cm = r1["cm"].reshape(NCORES, 128, NT, 8).transpose(0, 2, 1, 3) \
                 .reshape(NCORES, HALF, 8)
    score0_c = cm.max(axis=2)                                     # [8, 4096]
    cstar_c = cm.argmax(axis=2)                                   # [8, 4096]

    at2 = np.zeros((NCORES, D, 8 * PAD), np.float32)
    sg = np.full((NCORES, 128, NST), 1e30, np.float32)
    slot_of_row = np.full((NCORES, HALF), -1, np.int64)
    overflow = []                                                 # (core, row)
    for core in range(NCORES):
        for g in range(8):
            rows = np.nonzero(cstar_c[core] == g)[0]
            if len(rows) > PAD:
                overflow.extend((core, r) for r in rows[PAD:])
                rows = rows[:PAD]
            slots = g * PAD + np.arange(len(rows))
            slot_of_row[core, rows] = slots
            at2[core][:, slots] = a_slab[core][rows].T
            sg[core][slots % 128, slots // 128] = score0_c[core][rows]

    r2 = _run(ex2, {"at2": at2, "bt": bt_all, "sg": sg})
    within = r2["idx"][:, :, ::8]                                 # [8, 128, NST]

    match01 = np.empty((B, N), dtype=np.int32)
    score0 = np.empty((B, N), dtype=np.float32)
    valid = np.empty((B, N), dtype=bool)
    colmax = r1["colp"].reshape(B, 2, M).max(axis=1)              # [B, M]

    for core in range(NCORES):
        b, h = divmod(core, 2)
        s = score0_c[core]
        sl = slot_of_row[core]
        m = cstar_c[core] * 1024 + \
            within[core][sl % 128, sl // 128].astype(np.int64)
        sel = slice(h * HALF, (h + 1) * HALF)
        score0[b, sel] = s
        match01[b, sel] = m.astype(np.int32)
        valid[b, sel] = (s > 0.1) & (s == colmax[b][m])

    for core, row in overflow:                                    # ~never taken
        b, h = divmod(core, 2)
        simrow = a_slab[core][row] @ desc1[b].T
        n = h * HALF + row
        match01[b, n] = int(simrow.argmax())
        score0[b, n] = simrow.max()
        valid[b, n] = (score0[b, n] > 0.1) & \
                      (score0[b, n] == colmax[b][match01[b, n]])

    return match01, score0, valid
